# revision 1
# baseline (speedup 1.0000x reference)
"""Trainium2 Bass kernel for nn_GaussRegisterStep (B=4, T=2048, V=2048).

Strategy
--------
* The rfft / irfft in the reference are linear maps over the vocab dim, so
  they are fused into the weight matrices on the host:
      Wq = F @ qw.T, ...,  Wo = ow @ G * mem_out_scale * mem_scale, ...
  reducing the whole module to dense matmuls + a banded decaying attention
  + bias/gelu.
* rms_norm(x) @ W == diag(r) (x @ W), so the norm is applied as a per-token
  scaling of the projection outputs (r = rsqrt(mean(x^2)+eps)).
* decay = sigmoid(3) ~ 0.9526; decay^d < 4.2e-6 for d > 256, so the
  (anti-)causal decay attention is banded: each 512-token query group
  attends 6 x 128-token key blocks (window 768 >= 256+512).
* Sharding: 8 cores = (B=4) x (T in 2 halves of 1024 tokens). Each core
  gets its 1024 tokens plus a 256-token halo of x (zero padded past the
  sequence end); no collectives are needed.
* Everything on chip is V-major / C-major so no transposes are needed
  anywhere. Matmul operands are bitcast to float32r: full fp32 storage,
  the tensor engine runs them at ~bf16 rate with near-tf32 precision.
"""

import os
import numpy as np
from contextlib import ExitStack

# ---- problem constants (hardcoded per the task contract) -------------------
B, T, V, C, NF = 4, 2048, 2048, 1024, 512
P = 128
N_OWN = 1024           # tokens owned per core
HALO = 256             # future-token halo (band 256 <-> weight < 4.2e-6)
N_EXT = N_OWN + HALO   # 1280
VC = V // P            # 16 vocab chunks
CBN = C // P           # 8 channel blocks
SBK = N_EXT // P       # 10 s'-blocks
NR = 6                 # mask offsets per 512-token query group (window 768)
TGO = [(0, 512), (512, 512)]                 # owned token groups
TGE = [(0, 512), (512, 512), (1024, 256)]    # extended token groups
EPS = 1.1920929e-07
N_CORES = 8

_CACHE = {}
LAST_RESULTS = None  # test harness can read exec_time_ns from here


# ---------------------------------------------------------------------------
# host-side weight fusion
# ---------------------------------------------------------------------------
def _fuse_weights(qw, kw, vw, ow, decay_logit, mem_out_scale, freq_to_ch,
                  channel_mix, bias, ch_to_freq, op_out_scale, mem_scale,
                  op_scale):
    key = "F_G"
    if key not in _CACHE:
        v = np.arange(V, dtype=np.float64)[:, None]
        k = np.arange(1, NF + 1, dtype=np.float64)[None, :]
        ang = 2.0 * np.pi * v * k / V
        F = np.concatenate([np.cos(ang), -np.sin(ang)], axis=1)     # [V, 2n]
        G = np.concatenate([(2.0 / V) * np.cos(ang.T),
                            -(2.0 / V) * np.sin(ang.T)], axis=0)    # [2n, V]
        _CACHE[key] = (F, G)
    F, G = _CACHE[key]

    f64 = np.float64
    wq = (F @ qw.astype(f64).T).astype(np.float32)
    wk = (F @ kw.astype(f64).T).astype(np.float32)
    wv = (F @ vw.astype(f64).T).astype(np.float32)
    wo = (ow.astype(f64) @ G * float(mem_out_scale) *
          float(np.asarray(mem_scale).reshape(-1)[0])).astype(np.float32)
    wfc = ((F @ freq_to_ch.astype(f64).T) @ channel_mix.astype(f64)
           ).astype(np.float32)
    wr = (ch_to_freq.astype(f64).T @ G * float(op_out_scale) *
          float(np.asarray(op_scale).reshape(-1)[0])).astype(np.float32)

    decay = 1.0 / (1.0 + np.exp(-float(decay_logit)))
    # maskT[r][u, j]: weight for key at offset r*128+u vs query at offset j
    masks = np.zeros((NR, P, 512), dtype=np.float32)
    jj = np.arange(512, dtype=np.float64)[None, :]
    uu = np.arange(P, dtype=np.float64)[:, None]
    for r in range(NR):
        d = r * P + uu - jj
        with np.errstate(under="ignore"):
            w = np.where(d > 0, decay ** np.maximum(d - 1.0, 0.0), 0.0)
        masks[r] = w.astype(np.float32)

    biasc = np.ascontiguousarray(
        bias.astype(np.float32).reshape(CBN, P).T)          # [128, 8]
    return dict(wq=wq, wk=wk, wv=wv, wo=wo, wfc=wfc, wr=wr,
                masks=masks, biasc=biasc)


# ---------------------------------------------------------------------------
# bass program (identical on all 8 cores; data differs per core)
# ---------------------------------------------------------------------------
def _build_module():
    import concourse.bass as bass  # noqa: F401
    import concourse.mybir as mybir
    import concourse.tile as tile
    from concourse import bacc

    F32 = mybir.dt.float32
    F32R = mybir.dt.float32r
    AFT = mybir.ActivationFunctionType

    nc = bacc.Bacc("TRN2", target_bir_lowering=False, debug=False)

    xT = nc.dram_tensor("xT", [V, N_EXT], F32R, kind="ExternalInput").ap()
    wq = nc.dram_tensor("wq", [V, C], F32R, kind="ExternalInput").ap()
    wk = nc.dram_tensor("wk", [V, C], F32R, kind="ExternalInput").ap()
    wv = nc.dram_tensor("wv", [V, C], F32R, kind="ExternalInput").ap()
    wo = nc.dram_tensor("wo", [C, V], F32R, kind="ExternalInput").ap()
    wfc = nc.dram_tensor("wfc", [V, C], F32R, kind="ExternalInput").ap()
    wr = nc.dram_tensor("wr", [C, V], F32R, kind="ExternalInput").ap()
    masks = nc.dram_tensor("masks", [NR, P, 512], F32, kind="ExternalInput").ap()
    onesd = nc.dram_tensor("onesd", [P, P], F32R, kind="ExternalInput").ap()
    biasc = nc.dram_tensor("biasc", [P, CBN], F32, kind="ExternalInput").ap()
    yT = nc.dram_tensor("yT", [V, N_OWN], F32, kind="ExternalOutput").ap()

    def fr(ap):
        return ap.bitcast(F32R)

    def fv(ap):
        return ap.bitcast(F32)

    with tile.TileContext(nc) as tc:
        with ExitStack() as ctx:
            pp = ctx.enter_context(tc.tile_pool(name="ps", bufs=8, space="PSUM"))
            const = ctx.enter_context(tc.tile_pool(name="const", bufs=1))
            dpool = ctx.enter_context(tc.tile_pool(name="dram", bufs=1, space="DRAM"))

            # ---- constants --------------------------------------------------
            biasc_t = const.tile([P, CBN], F32, name="biasc", tag="biasc")
            nc.sync.dma_start(biasc_t[:], biasc)
            ones_t = const.tile([P, P], F32R, name="onest", tag="onest")
            nc.sync.dma_start(ones_t[:], onesd)
            ones_col = ones_t[:, 0:1]
            ones_row = ones_t[0:1, :]
            eps_t = const.tile([1, 1], F32, name="epst", tag="epst")
            nc.vector.memset(eps_t[:], EPS)

            # DRAM bounce buffers for q/k/v
            qd = dpool.tile([CBN, P, N_OWN], F32R, name="qd", tag="qd")
            kd = dpool.tile([CBN, P, N_EXT], F32R, name="kd", tag="kd")
            vd = dpool.tile([SBK, P, C], F32R, name="vd", tag="vd")
            rd = dpool.tile([CBN, P, N_OWN], F32R, name="rd", tag="rd")

            # ================= phase A: norms + q/k/v projections ===========
            with ExitStack() as pa:
                xtp = pa.enter_context(tc.tile_pool(name="xt", bufs=VC))
                sqp = pa.enter_context(tc.tile_pool(name="sq", bufs=2))
                rp = pa.enter_context(tc.tile_pool(name="rp", bufs=1))
                wp = pa.enter_context(tc.tile_pool(name="wp", bufs=32))
                evp = pa.enter_context(tc.tile_pool(name="ev", bufs=4))

                xt = []
                for vc in range(VC):
                    t = xtp.tile([P, N_EXT], F32R, name="xt", tag="xt")
                    nc.sync.dma_start(t[:], xT[vc * P:(vc + 1) * P, :])
                    xt.append(t)

                # sum of squares over V (via ones-matmul), then r = rsqrt
                ssp = [pp.tile([1, n], F32, name="ps", tag="ps") for (_, n) in TGE]
                for vc in range(VC):
                    sq = sqp.tile([P, N_EXT], F32R, name="sq", tag="sq")
                    nc.vector.tensor_mul(sq[:], fv(xt[vc][:]), fv(xt[vc][:]))
                    for g, (o, n) in enumerate(TGE):
                        nc.tensor.matmul(ssp[g][:], fr(ones_col),
                                         fr(sq[:, o:o + n]),
                                         start=(vc == 0), stop=(vc == VC - 1))
                mrow = rp.tile([1, N_EXT], F32, name="mrow", tag="mrow")
                for g, (o, n) in enumerate(TGE):
                    nc.scalar.activation(mrow[:, o:o + n], ssp[g][:],
                                         AFT.Identity, bias=eps_t[:], scale=1.0 / V)
                inv = rp.tile([1, N_EXT], F32, name="inv", tag="inv")
                nc.vector.reciprocal(inv[:], mrow[:])
                rrow = rp.tile([1, N_EXT], F32R, name="rrow", tag="rrow")
                nc.scalar.activation(rrow[:], inv[:], AFT.Sqrt)
                # r broadcast along partitions [128, N_EXT]
                rb = rp.tile([P, N_EXT], F32, name="rb", tag="rb")
                for (o, n) in TGE:
                    ps = pp.tile([P, n], F32, name="ps", tag="ps")
                    nc.tensor.matmul(ps[:], fr(ones_row),
                                     fr(rrow[:, o:o + n]), start=True, stop=True)
                    nc.vector.tensor_copy(rb[:, o:o + n], ps[:])
                # r as per-partition scalars [128, SBK]
                rc = rp.tile([P, SBK], F32, name="rc", tag="rc")
                for b in range(SBK):
                    nc.sync.dma_start(rc[:, b:b + 1], fv(rrow[0:1, b * P:(b + 1) * P]))

                # ---- q: [c, t_own] = sum_v Wq[v,c] * x[v,t], scaled by r ----
                for w_dram, bounce, tgl, scale_mode in (
                        (wq, qd, TGO, "rb"), (wk, kd, TGE, "rb")):
                    for cbh in range(2):
                        wt = []
                        for vc in range(VC):
                            t = wp.tile([P, 512], F32R, name="w", tag="w")
                            nc.sync.dma_start(
                                t[:], w_dram[vc * P:(vc + 1) * P,
                                             cbh * 512:(cbh + 1) * 512])
                            wt.append(t)
                        for cb4 in range(4):
                            cb = cbh * 4 + cb4
                            # split tg list so <=8 psum banks live
                            for tgs in (tgl[:2], tgl[2:]):
                                if not tgs:
                                    continue
                                pts = [pp.tile([P, n], F32, name="ps", tag="ps")
                                       for (_, n) in tgs]
                                for vc in range(VC):
                                    for gi, (o, n) in enumerate(tgs):
                                        nc.tensor.matmul(
                                            pts[gi][:],
                                            fr(wt[vc][:, cb4 * P:(cb4 + 1) * P]),
                                            fr(xt[vc][:, o:o + n]),
                                            start=(vc == 0), stop=(vc == VC - 1))
                                for gi, (o, n) in enumerate(tgs):
                                    ev = evp.tile([P, 512], F32R, name="ev", tag="ev")
                                    nc.vector.tensor_mul(
                                        ev[:, :n], pts[gi][:], rb[:, o:o + n])
                                    nc.sync.dma_start(
                                        bounce[cb, :, o:o + n], ev[:, :n])

                # ---- v: [s', c] = sum_v x[v,s'] * Wv[v,c], scaled by r ------
                for cg in range(2):
                    wt = []
                    for vc in range(VC):
                        t = wp.tile([P, 512], F32R, name="w", tag="w")
                        nc.sync.dma_start(
                            t[:], wv[vc * P:(vc + 1) * P,
                                     cg * 512:(cg + 1) * 512])
                        wt.append(t)
                    for sgrp in (range(0, 8), range(8, SBK)):
                        for sb in sgrp:
                            ps = pp.tile([P, 512], F32, name="ps", tag="ps")
                            for vc in range(VC):
                                nc.tensor.matmul(
                                    ps[:], fr(xt[vc][:, sb * P:(sb + 1) * P]),
                                    fr(wt[vc][:]),
                                    start=(vc == 0), stop=(vc == VC - 1))
                            ev = evp.tile([P, 512], F32R, name="ev", tag="ev")
                            nc.vector.tensor_scalar_mul(ev[:], ps[:],
                                                        rc[:, sb:sb + 1])
                            nc.sync.dma_start(vd[sb, :, cg * 512:(cg + 1) * 512],
                                              ev[:])

            # ================= phase B: banded decay attention ==============
            with ExitStack() as pb:
                kvp = pb.enter_context(tc.tile_pool(name="kv", bufs=1))
                qp = pb.enter_context(tc.tile_pool(name="qp", bufs=16))
                scp = pb.enter_context(tc.tile_pool(name="sc", bufs=12))

                masks_t = kvp.tile([P, NR * 512], F32, name="masks",
                                   tag="masks", bufs=1)
                for rr in range(NR):
                    nc.sync.dma_start(masks_t[:, rr * 512:(rr + 1) * 512],
                                      masks[rr])

                kt = []
                for cb in range(CBN):
                    t = kvp.tile([P, N_EXT], F32R, name="kt", tag="kt", bufs=CBN)
                    nc.sync.dma_start(t[:], kd[cb])
                    kt.append(t)
                vt = []
                for sb in range(SBK):
                    t = kvp.tile([P, C], F32R, name="vt", tag="vt", bufs=SBK)
                    nc.sync.dma_start(t[:], vd[sb])
                    vt.append(t)

                for tg, (o, n) in enumerate(TGO):
                    qt = []
                    for cb in range(CBN):
                        t = qp.tile([P, 512], F32R, name="qt", tag="qt")
                        nc.sync.dma_start(t[:], qd[cb, :, o:o + n])
                        qt.append(t)
                    scw = []
                    for rk in range(NR):
                        sb = tg * 4 + rk
                        ps = pp.tile([P, 512], F32, name="ps", tag="ps")
                        for cb in range(CBN):
                            nc.tensor.matmul(
                                ps[:], fr(kt[cb][:, sb * P:(sb + 1) * P]),
                                fr(qt[cb][:]),
                                start=(cb == 0), stop=(cb == CBN - 1))
                        sw = scp.tile([P, 512], F32R, name="sw", tag="sw")
                        nc.vector.tensor_mul(sw[:], ps[:],
                                             masks_t[:, rk * 512:(rk + 1) * 512])
                        scw.append(sw)
                    for cb in range(CBN):
                        ps = pp.tile([P, 512], F32, name="ps", tag="ps")
                        for rk in range(NR):
                            sb = tg * 4 + rk
                            nc.tensor.matmul(
                                ps[:], fr(vt[sb][:, cb * P:(cb + 1) * P]),
                                fr(scw[rk][:]),
                                start=(rk == 0), stop=(rk == NR - 1))
                        ev = scp.tile([P, 512], F32R, name="rev", tag="rev",
                                      bufs=4)
                        nc.vector.tensor_copy(ev[:], ps[:])
                        nc.sync.dma_start(rd[cb, :, o:o + n], ev[:])

            # ================= phase C: mem output + residual ===============
            # x2 outlives phase C -> pool at ctx level
            x2p = ctx.enter_context(tc.tile_pool(name="x2", bufs=VC))
            x2 = []
            with ExitStack() as pc:
                wop = pc.enter_context(tc.tile_pool(name="wo", bufs=CBN))
                rtp = pc.enter_context(tc.tile_pool(name="rt", bufs=CBN))
                retr = []
                for cb in range(CBN):
                    t = rtp.tile([P, N_OWN], F32R, name="retr", tag="retr")
                    nc.sync.dma_start(t[:], rd[cb])
                    retr.append(t)
                wot = []
                for cc in range(CBN):
                    t = wop.tile([P, V], F32R, name="wo", tag="wo")
                    nc.sync.dma_start(t[:], wo[cc * P:(cc + 1) * P, :])
                    wot.append(t)
                for vb in range(VC):
                    t = x2p.tile([P, N_OWN], F32R, name="x2", tag="x2")
                    nc.sync.dma_start(t[:], xT[vb * P:(vb + 1) * P, 0:N_OWN])
                    x2.append(t)
                for vb in range(VC):
                    for tg, (o, n) in enumerate(TGO):
                        ps = pp.tile([P, 512], F32, name="ps", tag="ps")
                        for cc in range(CBN):
                            nc.tensor.matmul(
                                ps[:], fr(wot[cc][:, vb * P:(vb + 1) * P]),
                                fr(retr[cc][:, o:o + n]),
                                start=(cc == 0), stop=(cc == CBN - 1))
                        nc.vector.tensor_add(x2[vb][:, o:o + n],
                                             fv(x2[vb][:, o:o + n]), ps[:])

            # ================= phase D: register op (mlp) ===================
            with ExitStack() as pd:
                hp = pd.enter_context(tc.tile_pool(name="hp", bufs=CBN))
                pd1 = pd.enter_context(ExitStack())
                sqp2 = pd1.enter_context(tc.tile_pool(name="sq2", bufs=2))
                rp2 = pd1.enter_context(tc.tile_pool(name="rp2", bufs=1))
                wfp = pd1.enter_context(tc.tile_pool(name="wf", bufs=32))
                evp2 = pd1.enter_context(tc.tile_pool(name="ev2", bufs=4))

                ssp2 = [pp.tile([1, n], F32, name="ps", tag="ps") for (_, n) in TGO]
                for vb in range(VC):
                    sq = sqp2.tile([P, N_OWN], F32R, name="sq2", tag="sq2")
                    nc.vector.tensor_mul(sq[:], fv(x2[vb][:]), fv(x2[vb][:]))
                    for g, (o, n) in enumerate(TGO):
                        nc.tensor.matmul(ssp2[g][:], fr(ones_col),
                                         fr(sq[:, o:o + n]),
                                         start=(vb == 0), stop=(vb == VC - 1))
                mrow2 = rp2.tile([1, N_OWN], F32, name="mrow2", tag="mrow2")
                for g, (o, n) in enumerate(TGO):
                    nc.scalar.activation(mrow2[:, o:o + n], ssp2[g][:],
                                         AFT.Identity, bias=eps_t[:], scale=1.0 / V)
                inv2 = rp2.tile([1, N_OWN], F32, name="inv2", tag="inv2")
                nc.vector.reciprocal(inv2[:], mrow2[:])
                rrow2 = rp2.tile([1, N_OWN], F32R, name="rrow2", tag="rrow2")
                nc.scalar.activation(rrow2[:], inv2[:], AFT.Sqrt)
                rb2 = rp2.tile([P, N_OWN], F32, name="rb2", tag="rb2")
                for (o, n) in TGO:
                    ps = pp.tile([P, n], F32, name="ps", tag="ps")
                    nc.tensor.matmul(ps[:], fr(ones_row),
                                     fr(rrow2[:, o:o + n]), start=True, stop=True)
                    nc.vector.tensor_copy(rb2[:, o:o + n], ps[:])

                ht = [hp.tile([P, N_OWN], F32R, name="h", tag="h") for _ in range(CBN)]
                for cbh in range(2):
                    wt = []
                    for vc in range(VC):
                        t = wfp.tile([P, 512], F32R, name="wf", tag="wf")
                        nc.sync.dma_start(
                            t[:], wfc[vc * P:(vc + 1) * P,
                                      cbh * 512:(cbh + 1) * 512])
                        wt.append(t)
                    for cb4 in range(4):
                        cb = cbh * 4 + cb4
                        pts = [pp.tile([P, n], F32, name="ps", tag="ps") for (_, n) in TGO]
                        for vc in range(VC):
                            for gi, (o, n) in enumerate(TGO):
                                nc.tensor.matmul(
                                    pts[gi][:],
                                    fr(wt[vc][:, cb4 * P:(cb4 + 1) * P]),
                                    fr(x2[vc][:, o:o + n]),
                                    start=(vc == 0), stop=(vc == VC - 1))
                        for gi, (o, n) in enumerate(TGO):
                            tmp = evp2.tile([P, 512], F32, name="tmp", tag="tmp")
                            nc.vector.tensor_mul(tmp[:], pts[gi][:],
                                                 rb2[:, o:o + n])
                            nc.scalar.activation(ht[cb][:, o:o + n], tmp[:],
                                                 AFT.Gelu,
                                                 bias=biasc_t[:, cb:cb + 1])

                pd1.close()
                # out2 + final residual + store
                with ExitStack() as pe:
                    wrp = pe.enter_context(tc.tile_pool(name="wr", bufs=CBN))
                    fip = pe.enter_context(tc.tile_pool(name="fi", bufs=6))
                    wrt = []
                    for cc in range(CBN):
                        t = wrp.tile([P, V], F32R, name="wr", tag="wr")
                        nc.sync.dma_start(t[:], wr[cc * P:(cc + 1) * P, :])
                        wrt.append(t)
                    for vb in range(VC):
                        for tg, (o, n) in enumerate(TGO):
                            ps = pp.tile([P, 512], F32, name="ps", tag="ps")
                            for cc in range(CBN):
                                nc.tensor.matmul(
                                    ps[:], fr(wrt[cc][:, vb * P:(vb + 1) * P]),
                                    fr(ht[cc][:, o:o + n]),
                                    start=(cc == 0), stop=(cc == CBN - 1))
                            fin = fip.tile([P, 512], F32, name="fin", tag="fin")
                            nc.vector.tensor_add(fin[:], fv(x2[vb][:, o:o + n]),
                                                 ps[:])
                            nc.sync.dma_start(yT[vb * P:(vb + 1) * P, o:o + n],
                                              fin[:])

    nc.compile()
    return nc


# ---------------------------------------------------------------------------
# entry point
# ---------------------------------------------------------------------------
def _round_tf32(a):
    b = np.ascontiguousarray(a, dtype=np.float32).view(np.uint32)
    b = (b + 0xFFF + ((b >> 13) & 1)) & np.uint32(0xFFFFE000)
    return b.view(np.float32)


def _prepare_in_maps(x, w):
    shared = {k: np.ascontiguousarray(v) for k, v in w.items()}
    shared["onesd"] = np.ones((P, P), dtype=np.float32)
    for k in ("wq", "wk", "wv", "wo", "wfc", "wr"):
        shared[k] = _round_tf32(shared[k])
    in_maps = []
    for core in range(N_CORES):
        b, h = core // 2, core % 2
        o = h * N_OWN
        n_real = min(N_EXT, T - o)
        xe = np.zeros((V, N_EXT), dtype=np.float32)
        xe[:, :n_real] = _round_tf32(x[b, o:o + n_real, :]).T
        m = dict(shared)
        m["xT"] = xe
        in_maps.append(m)
    return in_maps


def kernel(x, qw, kw, vw, ow, decay_logit, mem_out_scale, freq_to_ch,
           channel_mix, bias, ch_to_freq, op_out_scale, mem_scale, op_scale):
    global LAST_RESULTS
    from concourse.bass_utils import run_bass_kernel_spmd

    x = np.asarray(x, dtype=np.float32)
    w = _fuse_weights(qw, kw, vw, ow, decay_logit, mem_out_scale, freq_to_ch,
                      channel_mix, bias, ch_to_freq, op_out_scale, mem_scale,
                      op_scale)

    if "nc" not in _CACHE:
        _CACHE["nc"] = _build_module()
    nc = _CACHE["nc"]

    in_maps = _prepare_in_maps(x, w)

    trace = bool(int(os.environ.get("BASS_KERNEL_TRACE", "0")))
    res = run_bass_kernel_spmd(nc, in_maps, core_ids=list(range(N_CORES)),
                               trace=trace)
    LAST_RESULTS = res

    y = np.empty((B, T, V), dtype=np.float32)
    for core in range(N_CORES):
        b, h = core // 2, core % 2
        y[b, h * N_OWN:(h + 1) * N_OWN, :] = res.results[core]["yT"].T
    return y

